# revision 8
# baseline (speedup 1.0000x reference)
"""Bass/Trainium2 kernel for nn_MHSA_80461917323387.

Math (B=4, T=1024, D=1024, H=16, Dh=64; T==D makes the torch-style raw
reshape (B,T,D)->(B,H,Dh,T) equivalent to slicing the *sequence* dim):
  Q = x@Wq+bq; K = x@Wk+bk; V = x@Wv+bv           (each (B,1024,1024))
  per (b,h):  Qh = Q[b, 64h:64h+64, :]  (64x1024), same Kh, Vh
    A  = softmax_rows(Kh^T @ Vh * temp[h])        (1024x1024)
    out[b, 64h:64h+64, :] = Qh @ A

Sharding: 8 cores = 4 b x 2 head-groups (8 heads each). Each core gets
512 rows of x[b] (pre-transposed on host to xt = x-slice^T), full Wq/Wk/Wv,
and produces 512 rows of out[b]. No collectives.

On-chip layout per core:
  QT[t',r] = sum_c Wq[c,t'] xt[c,r] + bq[t']   8 tiles [128,512]  (lhsT for out-mm)
  K[r,t']  = sum_c xt[c,r] Wk[c,t'] + bk[t']   4 tiles [128,1024] (lhsT for scores)
  V[r,t']  likewise                             4 tiles [128,1024] (rhs for scores)
  scores(t-chunk) -> PSUM [128,1024]; exp via ACT (scale=temp, accum_out=rowsum)
  softmax normalization folded into the small QT slices (x 1/rowsum).
All matmuls run as float32r (full-rate fp32 path on trn2).
"""

import sys

sys.path.insert(0, "/opt/trn_rl_repo")

import numpy as np

import concourse.bass as bass
import concourse.bacc as bacc_mod
import concourse.mybir as mybir
from concourse.bass_utils import run_bass_kernel_spmd
from concourse.tile import TileContext

B, T, D, H = 4, 1024, 1024, 16
DH = D // H          # 64 rows per head-slice
HPC = 8              # heads per core
R = HPC * DH         # 512 rows per core
NC_CHUNKS = D // 128  # 8 contraction chunks
F32 = mybir.dt.float32
F32R = mybir.dt.float32r
AF = mybir.ActivationFunctionType


def build_nc() -> bass.Bass:
    nc = bacc_mod.Bacc(trn_type="TRN2")

    xt_h = nc.declare_dram_parameter("xt", [D, R], F32R, isOutput=False)
    wq_h = nc.declare_dram_parameter("wq", [D, D], F32R, isOutput=False)
    wk_h = nc.declare_dram_parameter("wk", [D, D], F32R, isOutput=False)
    wv_h = nc.declare_dram_parameter("wv", [D, D], F32R, isOutput=False)
    bqt_h = nc.declare_dram_parameter("bqt", [128, NC_CHUNKS], F32, isOutput=False)
    cv_h = nc.declare_dram_parameter("cvec", [1, 3 * D], F32R, isOutput=False)
    tmp_h = nc.declare_dram_parameter("tempv", [128, HPC], F32, isOutput=False)
    out_h = nc.declare_dram_parameter("out", [R, D], F32, isOutput=True)

    with TileContext(nc) as tc:
        with tc.tile_pool(name="const", bufs=1) as cpool, \
             tc.tile_pool(name="kv", bufs=1) as kvpool, \
             tc.tile_pool(name="qt", bufs=1) as qtpool:

            bqt = cpool.tile([128, NC_CHUNKS], F32, tag="bqt")
            tempv = cpool.tile([128, HPC], F32, tag="tempv")
            cvec = cpool.tile([1, 3 * D], F32R, tag="cvec")
            nc.sync.dma_start(out=bqt[:, :], in_=bqt_h[:, :])
            nc.sync.dma_start(out=tempv[:, :], in_=tmp_h[:, :])
            nc.sync.dma_start(out=cvec[:, :], in_=cv_h[:, :])
            bk1 = cvec[0:1, 0:D]
            bv1 = cvec[0:1, D:2 * D]
            ones = cvec[0:1, 2 * D:2 * D + 128]

            kt = [kvpool.tile([128, D], F32R, tag=f"k{i}", name=f"kt{i}") for i in range(4)]
            vt = [kvpool.tile([128, D], F32R, tag=f"v{i}", name=f"vt{i}") for i in range(4)]
            qt = [qtpool.tile([128, R], F32, tag=f"q{i}", name=f"qt{i}") for i in range(NC_CHUNKS)]

            # ---------- phase 1: projections ----------
            with tc.tile_pool(name="w", bufs=16) as wpool, \
                 tc.tile_pool(name="xt", bufs=8) as xtpool, \
                 tc.tile_pool(name="pj", bufs=2, space="PSUM") as pjpool, \
                 tc.tile_pool(name="pq", bufs=2, space="PSUM") as pqpool:

                xts = []
                for c in range(NC_CHUNKS):
                    t = xtpool.tile([128, R], F32R, tag="xt", name=f"xts{c}")
                    nc.sync.dma_start(out=t[:, :], in_=xt_h[c * 128:(c + 1) * 128, :])
                    xts.append(t)
                wqs = []
                for c in range(NC_CHUNKS):
                    t = wpool.tile([128, D], F32R, tag="w", name="wtile")
                    nc.sync.dma_start(out=t[:, :], in_=wq_h[c * 128:(c + 1) * 128, :])
                    wqs.append(t)
                wks = []
                for c in range(NC_CHUNKS):
                    t = wpool.tile([128, D], F32R, tag="w", name="wtile")
                    nc.sync.dma_start(out=t[:, :], in_=wk_h[c * 128:(c + 1) * 128, :])
                    wks.append(t)

                # QT projection: QT[t'c][:, r] ; bias bq via eviction ACT
                for tc_i in range(NC_CHUNKS):
                    pq = pqpool.tile([128, 512], F32, tag="pq", name="pq")
                    for c in range(NC_CHUNKS):
                        nc.tensor.matmul(
                            pq[:, :],
                            (wqs[c][:, tc_i * 128:(tc_i + 1) * 128]),
                            (xts[c][:, :]),
                            start=(c == 0), stop=(c == NC_CHUNKS - 1),
                        )
                    nc.scalar.activation(qt[tc_i][:, :], pq[:, :], AF.Identity,
                                         bias=bqt[:, tc_i:tc_i + 1])

                # K projection (+bk via K=1 ones-matmul), then V
                def proj_rows(w_tiles, bias_row, dst):
                    for rc in range(4):
                        pp = pjpool.tile([128, D], F32, tag="pj", name="pj")
                        for hf in range(2):
                            sl = slice(hf * 512, (hf + 1) * 512)
                            nc.tensor.matmul(pp[:, sl], ones,
                                             bias_row[:, sl],
                                             start=True, stop=False)
                            for c in range(NC_CHUNKS):
                                nc.tensor.matmul(
                                    pp[:, sl],
                                    (xts[c][:, rc * 128:(rc + 1) * 128]),
                                    (w_tiles[c][:, sl]),
                                    start=False, stop=(c == NC_CHUNKS - 1),
                                )
                        nc.scalar.activation(dst[rc][:, :], pp[:, :], AF.Copy)

                proj_rows(wks, bk1, kt)

                wvs = []
                for c in range(NC_CHUNKS):
                    t = wpool.tile([128, D], F32R, tag="w", name="wtile")
                    nc.sync.dma_start(out=t[:, :], in_=wv_h[c * 128:(c + 1) * 128, :])
                    wvs.append(t)
                proj_rows(wvs, bv1, vt)

            # ---------- phase 2: attention ----------
            with tc.tile_pool(name="a", bufs=16) as apool, \
                 tc.tile_pool(name="qts", bufs=16) as qtspool, \
                 tc.tile_pool(name="st", bufs=32) as stpool, \
                 tc.tile_pool(name="ob", bufs=2) as obpool, \
                 tc.tile_pool(name="ps", bufs=2, space="PSUM") as pspool, \
                 tc.tile_pool(name="po", bufs=2, space="PSUM") as popool:

                a_tiles = [[None] * NC_CHUNKS for _ in range(HPC)]
                qts_tiles = [[None] * NC_CHUNKS for _ in range(HPC)]

                def scores(j):
                    rc, p0 = j // 2, DH * (j % 2)
                    for t in range(NC_CHUNKS):
                        ps = pspool.tile([128, D], F32, tag="ps", name="ps")
                        lhs = kt[rc][p0:p0 + DH, t * 128:(t + 1) * 128]
                        for hf in range(2):
                            sl = slice(hf * 512, (hf + 1) * 512)
                            nc.tensor.matmul(ps[:, sl], (lhs),
                                             (vt[rc][p0:p0 + DH, sl]),
                                             start=True, stop=True)
                        at = apool.tile([128, D], F32R, tag="a", name="atile")
                        rs = stpool.tile([128, 1], F32, tag="rs", name="rs")
                        nc.scalar.activation(at[:, :], ps[:, :], AF.Exp,
                                             scale=tempv[:, j:j + 1],
                                             accum_out=rs[:, :])
                        rcp = stpool.tile([128, 1], F32, tag="rcp", name="rcp")
                        nc.vector.reciprocal(rcp[:, :], rs[:, :])
                        qs = qtspool.tile([128, DH], F32R, tag="qts", name="qts")
                        nc.vector.tensor_scalar_mul(
                            qs[:, :], qt[t][:, j * DH:(j + 1) * DH], rcp[:, :])
                        a_tiles[j][t] = at
                        qts_tiles[j][t] = qs

                def outmm(j):
                    po = popool.tile([64, D], F32, tag="po", name="po")
                    for t in range(NC_CHUNKS):
                        for hf in range(2):
                            sl = slice(hf * 512, (hf + 1) * 512)
                            nc.tensor.matmul(po[:, sl], (qts_tiles[j][t][:, :]),
                                             (a_tiles[j][t][:, sl]),
                                             start=(t == 0),
                                             stop=(t == NC_CHUNKS - 1))
                    ob = obpool.tile([64, D], F32, tag="ob", name="ob")
                    nc.scalar.activation(ob[:, :], po[:, :], AF.Copy)
                    nc.sync.dma_start(out=out_h[j * DH:(j + 1) * DH, :],
                                      in_=ob[:, :])
                    a_tiles[j] = [None] * NC_CHUNKS
                    qts_tiles[j] = [None] * NC_CHUNKS

                scores(0)
                scores(1)
                for j in range(2, HPC):
                    outmm(j - 2)
                    scores(j)
                outmm(HPC - 2)
                outmm(HPC - 1)

    nc.compile()
    return nc


_NC = None


def kernel(**inputs) -> np.ndarray:
    global _NC
    x = np.asarray(inputs["x"], np.float32)
    Wq = np.asarray(inputs["Wq"], np.float32)
    Wk = np.asarray(inputs["Wk"], np.float32)
    Wv = np.asarray(inputs["Wv"], np.float32)
    bq = np.asarray(inputs["bq"], np.float32)
    bk = np.asarray(inputs["bk"], np.float32)
    bv = np.asarray(inputs["bv"], np.float32)
    temp = np.asarray(inputs["temperature"], np.float32).reshape(H)

    if _NC is None:
        _NC = build_nc()

    bqt = np.ascontiguousarray(bq.reshape(NC_CHUNKS, 128).T)
    cvec = np.zeros((1, 3 * D), np.float32)
    cvec[0, 0:D] = bk
    cvec[0, D:2 * D] = bv
    cvec[0, 2 * D:] = 1.0
    in_maps = []
    for core in range(8):
        b, g = core // 2, core % 2
        xt = np.ascontiguousarray(x[b, g * R:(g + 1) * R, :].T)
        tempv = np.ascontiguousarray(
            np.broadcast_to(temp[g * HPC:(g + 1) * HPC][None, :], (128, HPC)))
        in_maps.append({
            "xt": xt, "wq": Wq, "wk": Wk, "wv": Wv,
            "bqt": bqt, "cvec": cvec, "tempv": tempv,
        })

    res = run_bass_kernel_spmd(_NC, in_maps, list(range(8)))
    out = np.empty((B, T, D), np.float32)
    for core in range(8):
        b, g = core // 2, core % 2
        out[b, g * R:(g + 1) * R, :] = res.results[core]["out"]
    return out
